# revision 12
# baseline (speedup 1.0000x reference)
"""Distributed memory-shard scale kernel for Trainium2 (8 NeuronCores).

Computes out[b, s, d] = x[b, s, d] * shards[shard_map[d], d] for
x: [4, 4096, 4096] f32, shards: [8, 4096] f32, shard_map: [4096] int.

Strategy: data-parallel over the flattened (batch*seq) rows — each of the
8 cores owns a contiguous 2048-row slice of x. The per-dim weight vector
w[d] = shards[shard_map[d], d] is gathered on the host (it is 16 KB; the
256 MB x-scaling stays on device) and passed to every core, so the device
preamble is just: load w row -> replicate to 128 partitions with K=1
outer-product matmuls (PE + DVE only).

The stream phase is HBM-bound: per-core DMA sustains ~433 GB/s
(f32 bytes, loads + stores combined; fabric spec 435, HBM stack spec
716 shared per core pair), so the 64 MiB each core moves floors at
~155 us plus ~11 us of fixed preamble/epilogue. Perfetto analysis of
earlier versions showed two schedule stalls this version removes: the
store queue idling ~30 us behind a slow on-device w build (fixed by
the host-side gather + early sync-ring w load), and a long store-only
tail because loads ran far ahead and finished early (fixed by the
small bufs=4 lead and 1 MB store granularity). Streaming: 16 tiles of
[128 rows, 4096 dims] (2 MB) loaded on the sync HWDGE ring, multiplied
by w in halves on DVE, stored as 1 MB halves on the scalar HWDGE ring
(first and last tiles in 512 KB quarters to shorten ramp and drain).

Note on run-to-run spread: the two cores of an HBM stack pair split
the stack's ~716 GB/s unevenly under contention (~407/305); whichever
core loses runs ~200-208 us while the winner runs ~170 us. That skew
is hardware arbitration, not kernel schedule — a bf16/SWDGE variant
that halved SBUF-side store bytes confirmed the ceiling is HBM-side,
and barrier/pacing schemes model out as net-negative.
"""

import numpy as np

import bass_rust as _bass_rust
import concourse.bass as bass
import concourse.tile as tile
from concourse import mybir
from concourse.bass_utils import run_bass_kernel_spmd

N_CORES = 8
BATCH, SEQ, DIM = 4, 4096, 4096
NUM_SHARDS = 8
ROWS_TOTAL = BATCH * SEQ               # 16384
ROWS_PER_CORE = ROWS_TOTAL // N_CORES  # 2048
P = 128                                # SBUF partitions
N_TILES = ROWS_PER_CORE // P           # 16 tiles of [128, 4096]
HW = DIM // 2                          # half width (2048)
QW = DIM // 4                          # quarter width (1024)
BUFS = 4

TRACE = False       # set True (e.g. from test.py) to capture an NTFF profile
LAST_RESULT = None  # BassKernelResults of the most recent kernel() call

_cached_nc = None


def _build_program() -> bass.Bass:
    f32 = mybir.dt.float32
    nc = bass.Bass()
    x_in = nc.dram_tensor("x", [ROWS_PER_CORE, DIM], f32, kind="ExternalInput")
    w_in = nc.dram_tensor("w", [1, DIM], f32, kind="ExternalInput")
    out = nc.dram_tensor("out", [ROWS_PER_CORE, DIM], f32,
                         kind="ExternalOutput")

    with tile.TileContext(nc) as tc:
        with tc.tile_pool(name="const", bufs=1) as cpool, \
             tc.tile_pool(name="xp", bufs=BUFS) as xpool:
            # ones row for the broadcast matmuls — engine op, no DMA
            ones = cpool.tile([1, P], f32)
            nc.vector.memset(ones[:], 1.0)
            # w row load FIRST on the sync ring: the scalar sequencer's
            # startup runs ~3 us behind sync, and w gates the first
            # mul+store — the 16 KB it costs the x stream is noise.
            w128 = cpool.tile([P, DIM], f32)
            wrow = w128[0:1, :]
            nc.sync.dma_start(wrow, w_in[:])
            # replicate w to all 128 partitions: ones[1,128].T @
            # wrow[1,512] -> PSUM[128,512], copy back on DVE.
            MMF = 512  # one PSUM bank per matmul
            with tc.tile_pool(name="ps", bufs=8, space="PSUM") as ppool:
                for k in range(DIM // MMF):
                    mm = ppool.tile([P, MMF], f32)
                    nc.tensor.matmul(mm[:], ones[:],
                                     w128[0:1, k * MMF:(k + 1) * MMF],
                                     start=True, stop=True)
                    nc.vector.tensor_copy(w128[:, k * MMF:(k + 1) * MMF],
                                          mm[:])

            # --- stream x through SBUF, scaling by w ---
            # [128, 8192] tiles, two consecutive rows per partition:
            # 32 KB-contiguous load descriptors measurably hold
            # ~433 GB/s bins vs ~428-430 for 16 KB ones. Stores keep
            # rev C's proven 1 MB quarter granularity on the scalar
            # ring. Column quarter q is row 2p + q//2, dims half q%2,
            # so each quarter multiplies against a w128 half. First and
            # last big tiles also LOAD in 1 MB quarters so stores start
            # early and the final load->mul->store chain stays short.
            x2v = x_in.rearrange("(i p t) d -> i p (t d)", p=P, t=2)
            o2v = out.rearrange("(i p t) d -> i p (t d)", p=P, t=2)
            N_BIG = ROWS_PER_CORE // (2 * P)   # 8
            for i in range(N_BIG):
                xt = xpool.tile([P, 2 * DIM], f32)
                edge = i == 0 or i == N_BIG - 1
                if not edge:
                    nc.sync.dma_start(xt[:], x2v[i])
                for q in range(4):
                    cols = slice(q * HW, (q + 1) * HW)
                    wcols = slice((q % 2) * HW, (q % 2 + 1) * HW)
                    if edge:
                        nc.sync.dma_start(xt[:, cols], x2v[i, :, cols])
                    nc.vector.tensor_mul(xt[:, cols], xt[:, cols],
                                         w128[:, wcols])
                    nc.scalar.dma_start(o2v[i, :, cols], xt[:, cols])
    # TRN2 allows one sync wait per instruction; split multi-wait
    # instructions the way bacc's compile pipeline does.
    _bass_rust.generate_event_semaphores(nc)
    return nc


def kernel(x, shards, shard_map):
    global _cached_nc, LAST_RESULT
    if _cached_nc is None:
        _cached_nc = _build_program()
    nc = _cached_nc

    x2 = np.asarray(x, dtype=np.float32).reshape(ROWS_TOTAL, DIM)
    sh = np.asarray(shards, dtype=np.float32)
    sm = np.asarray(shard_map).astype(np.int64)
    w = sh[sm, np.arange(DIM)].reshape(1, DIM).astype(np.float32)

    in_maps = [
        {"x": x2[c * ROWS_PER_CORE:(c + 1) * ROWS_PER_CORE], "w": w}
        for c in range(N_CORES)
    ]
    res = run_bass_kernel_spmd(nc, in_maps, core_ids=list(range(N_CORES)),
                               trace=TRACE)
    LAST_RESULT = res
    return np.concatenate([r["out"] for r in res.results],
                          axis=0).reshape(BATCH, SEQ, DIM)
